# revision 21
# baseline (speedup 1.0000x reference)
"""AnchorFreeGenerator centerness kernel for 8 TRN2 NeuronCores.

out[n] = max_m sqrt(l*r/max(l+r,eps)) * sqrt(t*b/max(t+b,eps))
with l,t,r,b the relu'd distances from point n to box m's sides.

Key algebra: for a valid box (w > 0), l*r/(l+r) = (px-x1)(x2-px)/w whenever
the point is inside horizontally, and the relu'd product is <= 0 otherwise.
So with u = (px-x1)(x2-px)/w (quadratic in px -> rank-3 in {px^2, px, 1}) and
v the y-side analogue:
    out[n] = sqrt(max(0, max_m u*relu(v)))
(u<0 or v<0 only happens outside the box; u*relu(v) is then <= 0; the
double-negative case u<0,v<0 is killed by the relu.)

Device mapping (per core, data-parallel over N):
  - TensorE: one K=32 bf16 matmul per 128-point tile produces [128, 512] PSUM
    = [u | v] for all 256 boxes. fp32-accurate via 3-way bf16 splits of the
    stationary/moving values (6 product slots per rank term).
  - ScalarE: w1 = relu(v), PSUM->SBUF.
  - VectorE: one custom-DVE op (Src0*Src1, accum=maxx) fusing multiply and
    max-reduce: vmax[:,t] = max(0, max_m u*w1).
  - ScalarE: final sqrt. Output DMA is contiguous per partition (p-major
    point permutation is done host-side when building the stationary).
Host-side numpy does only O(N)+O(M) input transforms (sharding, splits).
"""

import numpy as np
import ml_dtypes

import concourse.bacc as bacc
import concourse.mybir as mybir
import concourse.tile as tile
from concourse import dve_ops
from concourse.bass_utils import run_bass_kernel_spmd
from concourse.dve_ops import DveOp
from concourse.dve_spec import C0, Spec, Src0, Src1, lower, maxx, relu
from concourse.dve_uop import DveOpSpec

bf16 = ml_dtypes.bfloat16

N_TOTAL = 204800
N_CORES = 8
N_CORE = N_TOTAL // N_CORES  # 25600
M = 256  # boxes
K = 32  # stationary rows (27 used, padded)
NTILES = N_CORE // 128  # 200


# ---------------------------------------------------------------- custom op
def _ref_ttmax(in0, in1, c0, c1, c2):
    b = (in0.astype(np.float32) * np.maximum(np.nan_to_num(in1, nan=0.0), 0.0)).astype(np.float32)
    P = b.shape[0]
    acc = np.maximum(
        b.reshape(P, -1).max(axis=-1, keepdims=True),
        np.asarray(c0, np.float32).reshape(-1, 1),
    )
    return b, acc


def _make_ttmax_op():
    """Register (once) a custom DVE op: out=Src0*Src1, accum_out=max(C0, max out)."""
    name = "TT_RELU_MAX_REDUCE_ANT"
    for op in dve_ops.OPS:
        if op.name == name:
            return op
    spec = Spec(body=Src0 * relu(Src1), accum=maxx, accum_init=C0, reference=_ref_ttmax)
    shas = {}
    for ver in ("v3", "v4"):
        try:
            uops = lower(spec, ver=ver)
            shas[ver] = DveOpSpec(name=name, opcode=0, uops=uops, rd1_en=True).sha(ver)
        except Exception:
            pass
    op = DveOp(name, spec, subdim=False, uops_sha=shas)
    dve_ops.OPS.append(op)
    dve_ops._SUB_OPCODE_FOR_NAME[name] = max(dve_ops._SUB_OPCODE_FOR_NAME.values()) + 1
    dve_ops.CUSTOM_DVE_SPECS[name] = spec
    assert dve_ops._SUB_OPCODE_FOR_NAME[name] < 0x20
    return op


# ---------------------------------------------------------------- host prep
def _split3(v):
    """f64 array -> three bf16-exact f64 parts summing to ~v (2^-24-ish rel)."""
    b1 = v.astype(bf16).astype(np.float64)
    r1 = v - b1
    b2 = r1.astype(bf16).astype(np.float64)
    r2 = r1 - b2
    b3 = r2.astype(bf16).astype(np.float64)
    return b1, b2, b3


def _prep_host(points, gt_bboxes):
    """Build per-core stationary statT [8][K, N_CORE] and shared movB [K, 2M].

    Row k of the stationary pairs with row k of the moving matrix; the matmul
    accumulates sum_k stat[k,n]*mov[k,m] = [u | v] per (point, box).
    """
    px = points[:, 0].astype(np.float64)
    py = points[:, 1].astype(np.float64)
    x1, y1, x2, y2 = [gt_bboxes[:, i].astype(np.float64) for i in range(4)]

    c0x = (px.min() + px.max()) / 2.0
    c0y = (py.min() + py.max()) / 2.0
    if not np.isfinite(c0x):
        c0x = 0.0
    if not np.isfinite(c0y):
        c0y = 0.0
    pxc, pyc = px - c0x, py - c0y
    x1c, x2c, y1c, y2c = x1 - c0x, x2 - c0x, y1 - c0y, y2 - c0y

    w = x2c - x1c
    h = y2c - y1c
    bad_w = ~np.isfinite(w) | (w < 1e-6)
    bad_h = ~np.isfinite(h) | (h < 1e-6)
    rw = 1.0 / np.where(bad_w, 1.0, w)
    rh = 1.0 / np.where(bad_h, 1.0, h)

    # u = -pxc^2*rw + pxc*(rw*(x1c+x2c)) - rw*x1c*x2c ; degenerate box -> -1
    def _clean(a):
        return np.nan_to_num(a, nan=0.0, posinf=0.0, neginf=0.0)

    R = np.where(bad_w, 0.0, -rw)
    S = np.where(bad_w, 0.0, _clean(rw * (x1c + x2c)))
    Q = np.where(bad_w, -1.0, _clean(-rw * x1c * x2c))
    Rp = np.where(bad_h, 0.0, -rh)
    Sp = np.where(bad_h, 0.0, _clean(rh * (y1c + y2c)))
    Qp = np.where(bad_h, -1.0, _clean(-rh * y1c * y2c))

    A1, A2, A3 = _split3(pxc * pxc)
    B1, B2, B3 = _split3(pxc)
    D1, D2, D3 = _split3(pyc * pyc)
    E1, E2, E3 = _split3(pyc)
    R1, R2, R3 = _split3(R)
    S1, S2, S3 = _split3(S)
    Q1, Q2, Q3 = _split3(Q)
    Rp1, Rp2, Rp3 = _split3(Rp)
    Sp1, Sp2, Sp3 = _split3(Sp)
    Qp1, Qp2, Qp3 = _split3(Qp)
    onesN = np.ones_like(px)
    zeroM = np.zeros(M, np.float64)

    rows = [
        (A1, R1, zeroM), (A1, R2, zeroM), (A2, R1, zeroM),
        (A2, R2, zeroM), (A1, R3, zeroM), (A3, R1, zeroM),
        (B1, S1, zeroM), (B1, S2, zeroM), (B2, S1, zeroM),
        (B2, S2, zeroM), (B1, S3, zeroM), (B3, S1, zeroM),
        (onesN, Q1, Qp1), (onesN, Q2, Qp2), (onesN, Q3, Qp3),
        (D1, zeroM, Rp1), (D1, zeroM, Rp2), (D2, zeroM, Rp1),
        (D2, zeroM, Rp2), (D1, zeroM, Rp3), (D3, zeroM, Rp1),
        (E1, zeroM, Sp1), (E1, zeroM, Sp2), (E2, zeroM, Sp1),
        (E2, zeroM, Sp2), (E1, zeroM, Sp3), (E3, zeroM, Sp1),
    ]

    statT = np.zeros((K, N_TOTAL), bf16)
    movB = np.zeros((K, 2 * M), bf16)
    for k, (stat, mu, mv) in enumerate(rows):
        statT[k] = stat.astype(bf16)
        movB[k, :M] = mu.astype(bf16)
        movB[k, M:] = mv.astype(bf16)

    # Shard + permute: device column j = 128*t + i holds local point 200*i + t,
    # so vmax[i, t] DMAs back to out[200*i + t] contiguously per partition.
    i_idx = np.arange(128)[None, None, :]
    t_idx = np.arange(NTILES)[None, :, None]
    core0 = (np.arange(N_CORES) * N_CORE)[:, None, None]
    perm = (core0 + 200 * i_idx + t_idx).reshape(N_CORES, -1)  # [8, 25600]
    stat_shards = [np.ascontiguousarray(statT[:, perm[c]]) for c in range(N_CORES)]
    return stat_shards, movB


# ---------------------------------------------------------------- device
_NC_CACHE = {}


def _build_nc():
    if "nc" in _NC_CACHE:
        return _NC_CACHE["nc"]
    ttmax = _make_ttmax_op()
    nc = bacc.Bacc(target_bir_lowering=False)
    statT_d = nc.declare_dram_parameter("statT", [K, N_CORE], mybir.dt.bfloat16, isOutput=False)
    movB_d = nc.declare_dram_parameter("movB", [K, 2 * M], mybir.dt.bfloat16, isOutput=False)
    out_d = nc.declare_dram_parameter("out", [N_CORE], mybir.dt.float32, isOutput=True)

    NCHUNK = 8
    TCHUNK = NTILES // NCHUNK  # 25 tiles per chunk

    with tile.TileContext(nc) as tc:
        with (
            tc.tile_pool(name="const", bufs=1) as constp,
            tc.tile_pool(name="stat", bufs=NCHUNK) as statp,
            tc.tile_pool(name="w1", bufs=8) as w1p,
            tc.tile_pool(name="psum", bufs=8, space="PSUM") as psump,
        ):
            movB_sb = constp.tile([K, 2 * M], mybir.dt.bfloat16)
            nc.sync.dma_start(movB_sb[:], movB_d[:])
            vmax = constp.tile([128, NTILES], mybir.dt.float32)

            for c in range(NCHUNK):
                stat_sb = statp.tile([K, TCHUNK * 128], mybir.dt.bfloat16, tag="stat")
                nc.sync.dma_start(
                    stat_sb[:], statT_d[:, c * TCHUNK * 128 : (c + 1) * TCHUNK * 128]
                )
                for tl in range(TCHUNK):
                    t = c * TCHUNK + tl
                    ps = psump.tile([128, 2 * M], mybir.dt.float32)
                    nc.tensor.matmul(
                        ps[:],
                        stat_sb[:, tl * 128 : (tl + 1) * 128],
                        movB_sb[:],
                        start=True,
                        stop=True,
                    )
                    w1 = w1p.tile([128, M], mybir.dt.float32, tag="w1")
                    nc.scalar.activation(
                        w1[:], ps[:, M:], mybir.ActivationFunctionType.Copy
                    )
                    nc.vector._custom_dve(
                        ttmax,
                        out=ps[:, :M],
                        in0=ps[:, :M],
                        in1=w1[:],
                        s0=0.0,
                        accum_out=vmax[:, t : t + 1],
                    )

            outv = constp.tile([128, NTILES], mybir.dt.float32)
            nc.scalar.activation(outv[:], vmax[:], mybir.ActivationFunctionType.Sqrt)
            nc.sync.dma_start(out_d[:].rearrange("(i t) -> i t", i=128), outv[:])
    nc.compile()
    _NC_CACHE["nc"] = nc
    return nc


# ---------------------------------------------------------------- entry
def kernel(points, gt_bboxes, strides=None, _trace=False):
    assert points.shape == (N_TOTAL, 2) and gt_bboxes.shape == (M, 4)
    stat_shards, movB = _prep_host(np.asarray(points), np.asarray(gt_bboxes))
    nc = _build_nc()
    in_maps = [{"statT": stat_shards[c], "movB": movB} for c in range(N_CORES)]
    res = run_bass_kernel_spmd(nc, in_maps, core_ids=list(range(N_CORES)), trace=_trace)
    out = np.concatenate([res.results[c]["out"] for c in range(N_CORES)])
    if _trace:
        kernel._last_results = res
    return out.astype(np.float32)


if __name__ == "__main__":
    rng = np.random.default_rng(0)
    pts = (rng.random((N_TOTAL, 2)) * 1024).astype(np.float32)
    ctr = rng.random((M, 2)) * 1024
    wh = 16.0 + rng.random((M, 2)) * 240.0
    gt = np.concatenate([ctr - wh / 2, ctr + wh / 2], axis=-1).astype(np.float32)
    out = kernel(pts, gt, np.full((N_TOTAL,), 8.0, np.float32))
    print("out[:8]:", out[:8])
